# revision 3
# baseline (speedup 1.0000x reference)
"""ContactMapHead Trainium2 kernel (v6: dual-ring DMA + PE warmup).

Reference computation (per batch b):
    h = relu(X @ W^T + pb)            # [S, DP]
    scores = (h @ h^T) * cw + cb      # [S, S]  -- symmetric!

Sharding over 8 NeuronCores: core c handles batch b = c//2 with roll
offset off = (c%2)*1024 applied to X on the host. Each core computes
hT = relu(W @ XT + pb) for its full (rolled) batch, then emits the
circulant band of the symmetric score map: local tile rows i_t in 0..7
(tiles of 128), local cols j_t in i_t..i_t+8 (9 tiles of 128). Across
the two cores of a batch pair plus host-side transpose mirroring this
covers all 16x16 global tiles exactly.

v6 vs v5 (both host-transpose X/W to PE layout and run bf16):
- Input DMA is split across BOTH HWDGE rings (sync + scalar): one ring
  alone moves ~14 GB/s/engine, both together ~24 (measured), so dual
  issue nearly doubles input bandwidth (~220 -> ~380 GB/s).
- W^T is loaded first (split in two) so the first projection matmul
  isn't gated on a 0.5 MiB tail.
- Host pre-packs x/w in the exact [p, k, s] SBUF layout (no rearrange,
  contiguous partition lines).
- ~10 warm-up matmuls on an identity tile run while input streams, so
  the HAM clock-gate un-throttles (1.2 -> 2.4 GHz) before the real
  projection starts (the v5 trace showed the first 13 matmuls at half
  clock).
- Projection relu: pt0 on ScalarE, pt1 on VectorE (different PSUM
  banks -> truly parallel; v5 split one bank between both engines).
- Output rows alternate between the two rings to drain in parallel.
"""

import numpy as np
import ml_dtypes

from concourse import bacc, masks, mybir, tile

BF = ml_dtypes.bfloat16

P = 128
B, S, D = 4, 2048, 1024
DP = 256  # projection dim
NCORES = 8
KT = D // P  # 8 k-tiles over D
PT = DP // P  # 2 p-tiles over DP
SBLK = 512
HB = SBLK // 2
NSB = S // SBLK  # 4 s-blocks
NROW = 8  # local band rows (tiles of 128) per core
BANDW = 9 * P  # 1152 band columns per row
SEG = BANDW // 3  # 384-col band chunks
NWARM = 10

f32 = mybir.dt.float32
bf16 = mybir.dt.bfloat16


def _build_nc():
    nc = bacc.Bacc()
    xt = nc.declare_dram_parameter("xt", [P, KT, S], bf16, isOutput=False)
    wt = nc.declare_dram_parameter("wt", [P, KT, DP], bf16, isOutput=False)
    pb = nc.declare_dram_parameter("pb", [DP], f32, isOutput=False)
    cwb = nc.declare_dram_parameter("cwb", [2], f32, isOutput=False)
    out = nc.declare_dram_parameter("out", [NROW, P, BANDW], bf16, isOutput=True)

    with tile.TileContext(nc) as tc:
        _body(nc, tc, xt, wt, pb, cwb, out)
    nc.compile()
    return nc


def _body(nc, tc, xt, wt, pb, cwb, out):
    mult = mybir.AluOpType.mult
    add = mybir.AluOpType.add
    Relu = mybir.ActivationFunctionType.Relu
    Ident = mybir.ActivationFunctionType.Identity

    with (
        tc.tile_pool(name="const", bufs=1) as cpool,
        tc.tile_pool(name="orow", bufs=3) as opool,
        tc.tile_pool(name="pj", bufs=3, space="PSUM") as pj,
        tc.tile_pool(name="pw", bufs=3, space="PSUM") as pw,
        tc.tile_pool(name="pwarm", bufs=1, space="PSUM") as pwarm,
    ):
        # ---- PE warm-up: ~10 f32 matmuls (~430 ns each at cold clock)
        # on an identity tile built on-chip (no DMA dependency). These run
        # while the input streams in and trip the HAM activity monitor so
        # the real projection starts at 2.4 GHz.
        ident = cpool.tile([P, P], f32, tag="ident")
        masks.make_identity(nc, ident[:])
        wps = pwarm.tile([P, P], f32, tag="warm")
        for _ in range(NWARM):
            nc.tensor.matmul(wps[:], ident[:], ident[:], start=True, stop=True)

        # ---- constants (tiny, scalar ring) ----
        pb_t = cpool.tile([P, PT], f32, tag="pb_t")
        nc.scalar.dma_start(pb_t[:], pb.ap().rearrange("(t p) -> p t", p=P))

        cwb_t = cpool.tile([P, 2], f32, tag="cwb_t")
        nc.scalar.dma_start(cwb_t[:], cwb.ap().partition_broadcast(P))

        # ---- W^T first (split across both rings) ----
        wt_t = cpool.tile([P, KT, DP], bf16, tag="wt_t")
        nc.sync.dma_start(wt_t[:, 0 : KT // 2, :], wt.ap()[:, 0 : KT // 2, :])
        nc.scalar.dma_start(wt_t[:, KT // 2 : KT, :], wt.ap()[:, KT // 2 : KT, :])

        # ---- X^T streamed per s-block, each block split across rings ----
        xtile = cpool.tile([P, KT, S], bf16, tag="xtile")
        xv = xt.ap()
        for sb in range(NSB):
            c0 = sb * SBLK
            nc.sync.dma_start(
                xtile[:, :, c0 : c0 + HB], xv[:, :, c0 : c0 + HB]
            )
            nc.scalar.dma_start(
                xtile[:, :, c0 + HB : c0 + SBLK], xv[:, :, c0 + HB : c0 + SBLK]
            )

        # hT for the whole local map; relu writes per (pt, s-block) slices
        ht = cpool.tile([P, PT, S], bf16, tag="ht")

        def project(sb):
            o0 = sb * SBLK
            for pt in range(PT):
                pjs = pj.tile([P, SBLK], f32, tag="pj", name="pj")
                for k in range(KT):
                    nc.tensor.matmul(
                        pjs[:],
                        wt_t[:, k, pt * P : (pt + 1) * P],
                        xtile[:, k, o0 : o0 + SBLK],
                        start=(k == 0),
                        stop=(k == KT - 1),
                    )
                # pt0 -> ScalarE, pt1 -> VectorE: different PSUM banks, so
                # the two relu+bias passes run in parallel.
                if pt == 0:
                    nc.scalar.activation(
                        ht[:, pt, o0 : o0 + SBLK],
                        pjs[:],
                        Relu,
                        bias=pb_t[:, pt : pt + 1],
                    )
                else:
                    nc.vector.tensor_scalar(
                        ht[:, pt, o0 : o0 + SBLK],
                        pjs[:],
                        pb_t[:, pt : pt + 1],
                        0.0,
                        add,
                        mybir.AluOpType.max,
                    )

        def emit_pair_row(i_t):
            """Band row i_t: out[i_t] = cw * hT_i^T @ hT[band cols] + cb."""
            base = i_t * P
            psums = []
            for pt in range(PT):
                for si in range(3):
                    if pt == 0:
                        psums.append(pw.tile([P, SEG], f32, tag="pw", name="pw"))
                    c0 = base + si * SEG
                    nc.tensor.matmul(
                        psums[si][:],
                        ht[:, pt, base : base + P],
                        ht[:, pt, c0 : c0 + SEG],
                        start=(pt == 0),
                        stop=(pt == PT - 1),
                    )
            orow = opool.tile([P, BANDW], bf16, tag="orow", name="orow")
            for si in range(3):
                dst = orow[:, si * SEG : (si + 1) * SEG]
                if (i_t * 3 + si) % 2 == 0:
                    nc.vector.tensor_scalar(
                        dst, psums[si][:], cwb_t[:, 0:1], cwb_t[:, 1:2], mult, add
                    )
                else:
                    nc.scalar.activation(
                        dst, psums[si][:], Ident,
                        bias=cwb_t[:, 1:2], scale=cwb_t[:, 0:1],
                    )
            if i_t % 2 == 0:
                nc.sync.dma_start(out.ap()[i_t], orow[:])
            else:
                nc.scalar.dma_start(out.ap()[i_t], orow[:])

        project(0)
        project(1)
        project(2)
        # band rows 0..3 need hT cols up to 3*128+1152 = 1536 <= 3*SBLK
        for i_t in range(4):
            emit_pair_row(i_t)
        project(3)
        for i_t in range(4, NROW):
            emit_pair_row(i_t)


_NC_CACHE = None


def _get_nc():
    global _NC_CACHE
    if _NC_CACHE is None:
        _NC_CACHE = _build_nc()
    return _NC_CACHE


def _pack_pks(mat_T, rows, cols):
    """[rows*P, cols] -> [P, rows, cols] with d = k*P + p split as (k, p)."""
    return np.ascontiguousarray(
        mat_T.reshape(rows, P, cols).transpose(1, 0, 2)
    )


def _make_in_maps(hidden_states, proj_w, proj_b, clf_w, clf_b):
    hs = np.asarray(hidden_states, dtype=np.float32)
    wv = np.asarray(proj_w, dtype=np.float32)
    pbv = np.ascontiguousarray(np.asarray(proj_b, dtype=np.float32).reshape(DP))
    cwbv = np.array(
        [np.asarray(clf_w).reshape(-1)[0], np.asarray(clf_b).reshape(-1)[0]],
        dtype=np.float32,
    )
    wtv = _pack_pks(wv.astype(BF).T, KT, DP)  # [P, KT, DP]
    in_maps = []
    for b in range(B):
        xpks = _pack_pks(hs[b].astype(BF).T, KT, S)  # [P, KT, S]
        xpks_r = np.ascontiguousarray(np.roll(xpks, -S // 2, axis=2))
        for xv_ in (xpks, xpks_r):
            in_maps.append({"xt": xv_, "wt": wtv, "pb": pbv, "cwb": cwbv})
    return in_maps


def _assemble(results):
    scores = np.empty((B, S, S), np.float32)
    for c in range(NCORES):
        b, half = divmod(c, 2)
        o = np.asarray(results[c]["out"], dtype=np.float32)  # [NROW, P, BANDW]
        for i_t in range(NROW):
            gi = i_t + NROW * half
            strip = o[i_t]
            for lj in range(i_t, i_t + 9):
                gj = (lj + NROW * half) % 16
                V = strip[:, (lj - i_t) * P : (lj - i_t + 1) * P]
                scores[b, gi * P : (gi + 1) * P, gj * P : (gj + 1) * P] = V
                if gj != gi:
                    scores[b, gj * P : (gj + 1) * P, gi * P : (gi + 1) * P] = V.T
    return scores


def kernel(hidden_states, proj_w, proj_b, clf_w, clf_b):
    from concourse.bass_utils import run_bass_kernel_spmd

    nc = _get_nc()
    in_maps = _make_in_maps(hidden_states, proj_w, proj_b, clf_w, clf_b)
    res = run_bass_kernel_spmd(nc, in_maps, core_ids=list(range(NCORES)))
    return _assemble(res.results)


def run_traced(hidden_states, proj_w, proj_b, clf_w, clf_b):
    """Like kernel(), but also returns BassKernelResults with trace info."""
    from concourse.bass_utils import run_bass_kernel_spmd

    nc = _get_nc()
    in_maps = _make_in_maps(hidden_states, proj_w, proj_b, clf_w, clf_b)
    res = run_bass_kernel_spmd(
        nc, in_maps, core_ids=list(range(NCORES)), trace=True
    )
    return _assemble(res.results), res


# revision 7
# speedup vs baseline: 1.0775x; 1.0775x over previous
"""ContactMapHead Trainium2 kernel (v6: dual-ring DMA + PE warmup).

Reference computation (per batch b):
    h = relu(X @ W^T + pb)            # [S, DP]
    scores = (h @ h^T) * cw + cb      # [S, S]  -- symmetric!

Sharding over 8 NeuronCores: core c handles batch b = c//2 with roll
offset off = (c%2)*1024 applied to X on the host. Each core computes
hT = relu(W @ XT + pb) for its full (rolled) batch, then emits the
circulant band of the symmetric score map: local tile rows i_t in 0..7
(tiles of 128), local cols j_t in i_t..i_t+8 (9 tiles of 128). Across
the two cores of a batch pair plus host-side transpose mirroring this
covers all 16x16 global tiles exactly.

v6 vs v5 (both host-transpose X/W to PE layout and run bf16):
- Input DMA is split across BOTH HWDGE rings (sync + scalar): one ring
  alone moves ~14 GB/s/engine, both together ~24 (measured), so dual
  issue nearly doubles input bandwidth (~220 -> ~380 GB/s).
- W^T is loaded first (split in two) so the first projection matmul
  isn't gated on a 0.5 MiB tail.
- Host pre-packs x/w in the exact [p, k, s] SBUF layout (no rearrange,
  contiguous partition lines).
- Warm-up matmuls on an identity tile run while input streams, so the
  HAM clock-gate un-throttles (1.2 -> 2.4 GHz) before the real
  projection starts (the v5 trace showed the first 13 matmuls at half
  clock).
- Projection relu: pt0 on ScalarE, pt1 on VectorE (different PSUM
  banks -> truly parallel; v5 split one bank between both engines).
- Output rows alternate between the two rings to drain in parallel.

v7 vs v6 (trace: first proj MM still waited until 20.8 us):
- pb/cwb DMAs moved to the gpsimd SWDGE ring: on the scalar HWDGE ring
  each tiny DMA cost ~1 us of ring time and delayed wt_b/sb0b by ~5 us.
- First projection chunk is 256 cols (two half-chunks, one per ring),
  so the PE starts as soon as ~0.75 MiB has landed instead of 1.5 MiB.
- Warm-up sized (11 MMs) to end right when the first chunk lands.
- PSUM rebalance: pj 2 / pw 5 / warm 1 banks. With pw=3, band row k+1
  had to wait for ALL of row k's copies (v6 showed ~4 us of band-phase
  stalls); with 5 bufs two rows' matmuls/copies overlap.
- Tail: rows 6/7 DMA out per 384-col segment as each copy finishes.
"""

import numpy as np
import ml_dtypes

from concourse import bacc, masks, mybir, tile

BF = ml_dtypes.bfloat16

P = 128
B, S, D = 4, 2048, 1024
DP = 256  # projection dim
NCORES = 8
KT = D // P  # 8 k-tiles over D
PT = DP // P  # 2 p-tiles over DP
SBLK = 512
HB = SBLK // 2
NSB = S // SBLK  # 4 s-blocks
NROW = 8  # local band rows (tiles of 128) per core
BANDW = 9 * P  # 1152 band columns per row
SEG = BANDW // 3  # 384-col band chunks
NWARM = 11
# projection chunks: a small opener so the PE starts early, then 512s
PCHUNKS = [(0, 256), (256, 512), (512, 1024), (1024, 1536), (1536, 2048)]

f32 = mybir.dt.float32
bf16 = mybir.dt.bfloat16


def _build_nc():
    nc = bacc.Bacc()
    xt = nc.declare_dram_parameter("xt", [P, KT, S], bf16, isOutput=False)
    wt = nc.declare_dram_parameter("wt", [P, KT, DP], bf16, isOutput=False)
    pb = nc.declare_dram_parameter("pb", [DP], f32, isOutput=False)
    cwb = nc.declare_dram_parameter("cwb", [2], f32, isOutput=False)
    out = nc.declare_dram_parameter("out", [NROW, P, BANDW], bf16, isOutput=True)

    with tile.TileContext(nc) as tc:
        _body(nc, tc, xt, wt, pb, cwb, out)
    nc.compile()
    return nc


def _body(nc, tc, xt, wt, pb, cwb, out):
    mult = mybir.AluOpType.mult
    add = mybir.AluOpType.add
    Relu = mybir.ActivationFunctionType.Relu
    Ident = mybir.ActivationFunctionType.Identity

    with (
        tc.tile_pool(name="const", bufs=1) as cpool,
        tc.tile_pool(name="orow", bufs=3) as opool,
        tc.tile_pool(name="pj", bufs=2, space="PSUM") as pj,
        tc.tile_pool(name="pw", bufs=5, space="PSUM") as pw,
        tc.tile_pool(name="pwarm", bufs=1, space="PSUM") as pwarm,
    ):
        # ---- PE warm-up: f32 matmuls (~430 ns each at cold clock) on an
        # identity tile built on-chip (no DMA dependency). These run
        # while the input streams in and trip the HAM activity monitor so
        # the real projection starts at 2.4 GHz.
        ident = cpool.tile([P, P], f32, tag="ident")
        masks.make_identity(nc, ident[:])
        wps = pwarm.tile([P, P], f32, tag="warm")
        for _ in range(NWARM):
            nc.tensor.matmul(wps[:], ident[:], ident[:], start=True, stop=True)

        # ---- constants on the gpsimd SWDGE ring (tiny DMAs cost ~1 us
        # of HWDGE ring time each, so keep them off the input rings) ----
        pb_t = cpool.tile([P, PT], f32, tag="pb_t")
        nc.gpsimd.dma_start(pb_t[:], pb.ap().rearrange("(t p) -> p t", p=P))

        cwb_t = cpool.tile([P, 2], f32, tag="cwb_t")
        nc.gpsimd.dma_start(cwb_t[:], cwb.ap().partition_broadcast(P))

        # ---- W^T first on each ring, then X^T half-chunks ----
        wt_t = cpool.tile([P, KT, DP], bf16, tag="wt_t")
        nc.sync.dma_start(wt_t[:, 0 : KT // 2, :], wt.ap()[:, 0 : KT // 2, :])
        nc.scalar.dma_start(wt_t[:, KT // 2 : KT, :], wt.ap()[:, KT // 2 : KT, :])

        xtile = cpool.tile([P, KT, S], bf16, tag="xtile")
        xv = xt.ap()
        for sb in range(NSB):
            c0 = sb * SBLK
            nc.sync.dma_start(
                xtile[:, :, c0 : c0 + HB], xv[:, :, c0 : c0 + HB]
            )
            nc.scalar.dma_start(
                xtile[:, :, c0 + HB : c0 + SBLK], xv[:, :, c0 + HB : c0 + SBLK]
            )

        # hT for the whole local map; relu writes per (pt, chunk) slices
        ht = cpool.tile([P, PT, S], bf16, tag="ht")

        def project(c0, c1):
            w = c1 - c0
            for pt in range(PT):
                pjs = pj.tile([P, SBLK], f32, tag="pj", name="pj")
                for k in range(KT):
                    nc.tensor.matmul(
                        pjs[:, 0:w],
                        wt_t[:, k, pt * P : (pt + 1) * P],
                        xtile[:, k, c0:c1],
                        start=(k == 0),
                        stop=(k == KT - 1),
                    )
                # pt0 -> ScalarE, pt1 -> VectorE: different PSUM banks, so
                # the two relu+bias passes run in parallel.
                if pt == 0:
                    nc.scalar.activation(
                        ht[:, pt, c0:c1],
                        pjs[:, 0:w],
                        Relu,
                        bias=pb_t[:, pt : pt + 1],
                    )
                else:
                    nc.vector.tensor_scalar(
                        ht[:, pt, c0:c1],
                        pjs[:, 0:w],
                        pb_t[:, pt : pt + 1],
                        0.0,
                        add,
                        mybir.AluOpType.max,
                    )

        def emit_pair_row(i_t):
            """Band row i_t: out[i_t] = cw * hT_i^T @ hT[band cols] + cb."""
            base = i_t * P
            psums = []
            for pt in range(PT):
                for si in range(3):
                    if pt == 0:
                        psums.append(pw.tile([P, SEG], f32, tag="pw", name="pw"))
                    c0 = base + si * SEG
                    nc.tensor.matmul(
                        psums[si][:],
                        ht[:, pt, base : base + P],
                        ht[:, pt, c0 : c0 + SEG],
                        start=(pt == 0),
                        stop=(pt == PT - 1),
                    )
            orow = opool.tile([P, BANDW], bf16, tag="orow", name="orow")
            tail = i_t >= NROW - 2
            for si in range(3):
                dst = orow[:, si * SEG : (si + 1) * SEG]
                if (i_t * 3 + si) % 2 == 0:
                    nc.vector.tensor_scalar(
                        dst, psums[si][:], cwb_t[:, 0:1], cwb_t[:, 1:2], mult, add
                    )
                else:
                    nc.scalar.activation(
                        dst, psums[si][:], Ident,
                        bias=cwb_t[:, 1:2], scale=cwb_t[:, 0:1],
                    )
                if tail:
                    # last rows: drain per segment so the final DMA is small
                    eng = nc.sync if (i_t + si) % 2 == 0 else nc.scalar
                    eng.dma_start(
                        out.ap()[i_t][:, si * SEG : (si + 1) * SEG], dst
                    )
            if not tail:
                if i_t % 2 == 0:
                    nc.sync.dma_start(out.ap()[i_t], orow[:])
                else:
                    nc.scalar.dma_start(out.ap()[i_t], orow[:])

        for c0, c1 in PCHUNKS[:4]:
            project(c0, c1)
        # band rows 0..3 need hT cols up to 3*128+1152 = 1536
        for i_t in range(4):
            emit_pair_row(i_t)
        project(*PCHUNKS[4])
        for i_t in range(4, NROW):
            emit_pair_row(i_t)


_NC_CACHE = None


def _get_nc():
    global _NC_CACHE
    if _NC_CACHE is None:
        _NC_CACHE = _build_nc()
    return _NC_CACHE


def _pack_pks(mat_T, rows, cols):
    """[rows*P, cols] -> [P, rows, cols] with d = k*P + p split as (k, p)."""
    return np.ascontiguousarray(
        mat_T.reshape(rows, P, cols).transpose(1, 0, 2)
    )


def _make_in_maps(hidden_states, proj_w, proj_b, clf_w, clf_b):
    hs = np.asarray(hidden_states, dtype=np.float32)
    wv = np.asarray(proj_w, dtype=np.float32)
    pbv = np.ascontiguousarray(np.asarray(proj_b, dtype=np.float32).reshape(DP))
    cwbv = np.array(
        [np.asarray(clf_w).reshape(-1)[0], np.asarray(clf_b).reshape(-1)[0]],
        dtype=np.float32,
    )
    wtv = _pack_pks(wv.astype(BF).T, KT, DP)  # [P, KT, DP]
    in_maps = []
    for b in range(B):
        xpks = _pack_pks(hs[b].astype(BF).T, KT, S)  # [P, KT, S]
        xpks_r = np.ascontiguousarray(np.roll(xpks, -S // 2, axis=2))
        for xv_ in (xpks, xpks_r):
            in_maps.append({"xt": xv_, "wt": wtv, "pb": pbv, "cwb": cwbv})
    return in_maps


def _assemble(results):
    scores = np.empty((B, S, S), np.float32)
    for c in range(NCORES):
        b, half = divmod(c, 2)
        o = np.asarray(results[c]["out"], dtype=np.float32)  # [NROW, P, BANDW]
        for i_t in range(NROW):
            gi = i_t + NROW * half
            strip = o[i_t]
            for lj in range(i_t, i_t + 9):
                gj = (lj + NROW * half) % 16
                V = strip[:, (lj - i_t) * P : (lj - i_t + 1) * P]
                scores[b, gi * P : (gi + 1) * P, gj * P : (gj + 1) * P] = V
                if gj != gi:
                    scores[b, gj * P : (gj + 1) * P, gi * P : (gi + 1) * P] = V.T
    return scores


def kernel(hidden_states, proj_w, proj_b, clf_w, clf_b):
    from concourse.bass_utils import run_bass_kernel_spmd

    nc = _get_nc()
    in_maps = _make_in_maps(hidden_states, proj_w, proj_b, clf_w, clf_b)
    res = run_bass_kernel_spmd(nc, in_maps, core_ids=list(range(NCORES)))
    return _assemble(res.results)


def run_traced(hidden_states, proj_w, proj_b, clf_w, clf_b):
    """Like kernel(), but also returns BassKernelResults with trace info."""
    from concourse.bass_utils import run_bass_kernel_spmd

    nc = _get_nc()
    in_maps = _make_in_maps(hidden_states, proj_w, proj_b, clf_w, clf_b)
    res = run_bass_kernel_spmd(
        nc, in_maps, core_ids=list(range(NCORES)), trace=True
    )
    return _assemble(res.results), res


# revision 8
# speedup vs baseline: 1.1280x; 1.0468x over previous
"""ContactMapHead Trainium2 kernel (v8: fine-grained chunk pipeline).

Reference computation (per batch b):
    h = relu(X @ W^T + pb)            # [S, DP]
    scores = (h @ h^T) * cw + cb      # [S, S]  -- symmetric!

Sharding over 8 NeuronCores: core c handles batch b = c//2 with roll
offset off = (c%2)*1024 applied to X on the host. Each core computes
hT = relu(W @ XT + pb) for its full (rolled) batch, then emits the
circulant band of the symmetric score map: local tile rows i_t in 0..7
(tiles of 128), local cols j_t in i_t..i_t+8 (9 tiles of 128). Across
the two cores of a batch pair plus host-side transpose mirroring this
covers all 16x16 global tiles exactly.

Design history (per-core timeline facts from ntff traces):
- v5: host-transpose X/W + bf16 everywhere (2e-2 tolerance, bf16 lands
  at 4e-3): kills all 144 PE transposes and halves DMA. 66.5->52.7us.
- v6/v7: dual-HWDGE-ring input (one ring ~220 GB/s, both ~331 = HBM
  cap), wt first, PE warm-up matmuls (HAM un-throttle), PSUM
  rebalance. ->48.4us.
- v8: the PE phase is the floor (warm stream = 64 proj MM x 213ns +
  48 band MM x 162ns = 21.4us). Everything else must hide behind it:
  * projection in 8 x 256-col chunks, chunk i alternating rings, so
    the PE starts on the first 0.75 MiB (~13us) and ANY ready chunk
    keeps it busy (the v7 Tile schedule stalled 4.2us waiting for the
    second chunk while the first sat ready).
  * band rows dovetail between projection chunks as their last hT
    column arrives (row i needs cols < i*128+1152).
  * band copies were the v7 band bottleneck (24 x ~670ns on 2 engines
    + out-DMA issue stealing ScalarE time): out-DMAs now issue from
    sync/gpsimd only, one orow buffer per row (no recycle wait), 6
    band PSUM banks so two rows overlap, warm-up reuses a band bank.
  * last two rows drain per 384-col segment to cut the final-DMA tail.
"""

import numpy as np
import ml_dtypes

from concourse import bacc, masks, mybir, tile

BF = ml_dtypes.bfloat16

P = 128
B, S, D = 4, 2048, 1024
DP = 256  # projection dim
NCORES = 8
KT = D // P  # 8 k-tiles over D
PT = DP // P  # 2 p-tiles over DP
CHK = 256  # projection/DMA chunk width (s columns)
NCH = S // CHK  # 8 chunks
NROW = 8  # local band rows (tiles of 128) per core
BANDW = 9 * P  # 1152 band columns per row
SEG = BANDW // 3  # 384-col band chunks
NWARM = 11

f32 = mybir.dt.float32
bf16 = mybir.dt.bfloat16


def _build_nc():
    nc = bacc.Bacc()
    xt = nc.declare_dram_parameter("xt", [P, KT, S], bf16, isOutput=False)
    wt = nc.declare_dram_parameter("wt", [P, KT, DP], bf16, isOutput=False)
    pb = nc.declare_dram_parameter("pb", [DP], f32, isOutput=False)
    cwb = nc.declare_dram_parameter("cwb", [2], f32, isOutput=False)
    out = nc.declare_dram_parameter("out", [NROW, P, BANDW], bf16, isOutput=True)

    with tile.TileContext(nc) as tc:
        _body(nc, tc, xt, wt, pb, cwb, out)
    nc.compile()
    return nc


def _body(nc, tc, xt, wt, pb, cwb, out):
    mult = mybir.AluOpType.mult
    add = mybir.AluOpType.add
    Relu = mybir.ActivationFunctionType.Relu
    Ident = mybir.ActivationFunctionType.Identity

    with (
        tc.tile_pool(name="const", bufs=1) as cpool,
        tc.tile_pool(name="orow", bufs=NROW) as opool,
        tc.tile_pool(name="pj", bufs=2, space="PSUM") as pj,
        tc.tile_pool(name="pw", bufs=6, space="PSUM") as pw,
    ):
        # ---- PE warm-up: f32 matmuls on an on-chip identity (no DMA
        # dependency). They run while the input streams in and trip the
        # HAM activity monitor so real work starts at 2.4 GHz. The psum
        # comes from the band pool and is recycled long before row 0.
        ident = cpool.tile([P, P], f32, tag="ident")
        masks.make_identity(nc, ident[:])
        wps = pw.tile([P, SEG], f32, tag="pw", name="warm")
        for _ in range(NWARM):
            nc.tensor.matmul(wps[:, 0:P], ident[:], ident[:], start=True, stop=True)

        # ---- constants on the gpsimd SWDGE ring (tiny DMAs cost ~1 us
        # of HWDGE ring time each, so keep them off the input rings) ----
        pb_t = cpool.tile([P, PT], f32, tag="pb_t")
        nc.gpsimd.dma_start(pb_t[:], pb.ap().rearrange("(t p) -> p t", p=P))

        cwb_t = cpool.tile([P, 2], f32, tag="cwb_t")
        nc.gpsimd.dma_start(cwb_t[:], cwb.ap().partition_broadcast(P))

        # ---- input: wt whole on sync; x chunks alternate scalar/sync so
        # both rings stream (one ring ~220 GB/s, both ~330 = HBM cap) and
        # chunks complete in index order with ~1us spacing.
        wt_t = cpool.tile([P, KT, DP], bf16, tag="wt_t")
        nc.sync.dma_start(wt_t[:], wt.ap()[:])

        xtile = cpool.tile([P, KT, S], bf16, tag="xtile")
        xv = xt.ap()
        for ch in range(NCH):
            c0 = ch * CHK
            eng = nc.scalar if ch % 2 == 0 else nc.sync
            eng.dma_start(xtile[:, :, c0 : c0 + CHK], xv[:, :, c0 : c0 + CHK])

        # hT for the whole local map; relu writes per (pt, chunk) slices
        ht = cpool.tile([P, PT, S], bf16, tag="ht")

        def project(ch):
            c0 = ch * CHK
            for pt in range(PT):
                pjs = pj.tile([P, 512], f32, tag="pj", name="pj")
                for k in range(KT):
                    nc.tensor.matmul(
                        pjs[:, 0:CHK],
                        wt_t[:, k, pt * P : (pt + 1) * P],
                        xtile[:, k, c0 : c0 + CHK],
                        start=(k == 0),
                        stop=(k == KT - 1),
                    )
                # pt0 -> ScalarE, pt1 -> VectorE: different PSUM banks, so
                # the two relu+bias passes run in parallel.
                if pt == 0:
                    nc.scalar.activation(
                        ht[:, pt, c0 : c0 + CHK],
                        pjs[:, 0:CHK],
                        Relu,
                        bias=pb_t[:, pt : pt + 1],
                    )
                else:
                    nc.vector.tensor_scalar(
                        ht[:, pt, c0 : c0 + CHK],
                        pjs[:, 0:CHK],
                        pb_t[:, pt : pt + 1],
                        0.0,
                        add,
                        mybir.AluOpType.max,
                    )

        def emit_pair_row(i_t):
            """Band row i_t: out[i_t] = cw * hT_i^T @ hT[band cols] + cb."""
            base = i_t * P
            psums = []
            for pt in range(PT):
                for si in range(3):
                    if pt == 0:
                        psums.append(pw.tile([P, SEG], f32, tag="pw", name="pw"))
                    c0 = base + si * SEG
                    nc.tensor.matmul(
                        psums[si][:],
                        ht[:, pt, base : base + P],
                        ht[:, pt, c0 : c0 + SEG],
                        start=(pt == 0),
                        stop=(pt == PT - 1),
                    )
            orow = opool.tile([P, BANDW], bf16, tag="orow", name="orow")
            tail = i_t >= NROW - 2
            for si in range(3):
                dst = orow[:, si * SEG : (si + 1) * SEG]
                if (i_t * 3 + si) % 2 == 0:
                    nc.vector.tensor_scalar(
                        dst, psums[si][:], cwb_t[:, 0:1], cwb_t[:, 1:2], mult, add
                    )
                else:
                    nc.scalar.activation(
                        dst, psums[si][:], Ident,
                        bias=cwb_t[:, 1:2], scale=cwb_t[:, 0:1],
                    )
                if tail:
                    # last rows: drain per segment so the final DMA is small
                    eng = nc.sync if (i_t + si) % 2 == 0 else nc.gpsimd
                    eng.dma_start(
                        out.ap()[i_t][:, si * SEG : (si + 1) * SEG], dst
                    )
            if not tail:
                # out-DMAs issue from sync/gpsimd: a DMA issue costs ~600ns
                # on the issuing engine and ScalarE is busy with copies
                eng = nc.sync if i_t % 2 == 0 else nc.gpsimd
                eng.dma_start(out.ap()[i_t], orow[:])

        # dovetail: emit each band row right after the chunk that
        # completes its rhs span (row i needs hT cols < i*128+1152)
        for ch in range(5):
            project(ch)
        emit_pair_row(0)
        emit_pair_row(1)
        project(5)
        emit_pair_row(2)
        emit_pair_row(3)
        project(6)
        emit_pair_row(4)
        emit_pair_row(5)
        project(7)
        emit_pair_row(6)
        emit_pair_row(7)


_NC_CACHE = None


def _get_nc():
    global _NC_CACHE
    if _NC_CACHE is None:
        _NC_CACHE = _build_nc()
    return _NC_CACHE


def _pack_pks(mat_T, rows, cols):
    """[rows*P, cols] -> [P, rows, cols] with d = k*P + p split as (k, p)."""
    return np.ascontiguousarray(
        mat_T.reshape(rows, P, cols).transpose(1, 0, 2)
    )


def _make_in_maps(hidden_states, proj_w, proj_b, clf_w, clf_b):
    hs = np.asarray(hidden_states, dtype=np.float32)
    wv = np.asarray(proj_w, dtype=np.float32)
    pbv = np.ascontiguousarray(np.asarray(proj_b, dtype=np.float32).reshape(DP))
    cwbv = np.array(
        [np.asarray(clf_w).reshape(-1)[0], np.asarray(clf_b).reshape(-1)[0]],
        dtype=np.float32,
    )
    wtv = _pack_pks(wv.astype(BF).T, KT, DP)  # [P, KT, DP]
    in_maps = []
    for b in range(B):
        xpks = _pack_pks(hs[b].astype(BF).T, KT, S)  # [P, KT, S]
        xpks_r = np.ascontiguousarray(np.roll(xpks, -S // 2, axis=2))
        for xv_ in (xpks, xpks_r):
            in_maps.append({"xt": xv_, "wt": wtv, "pb": pbv, "cwb": cwbv})
    return in_maps


def _assemble(results):
    scores = np.empty((B, S, S), np.float32)
    for c in range(NCORES):
        b, half = divmod(c, 2)
        o = np.asarray(results[c]["out"], dtype=np.float32)  # [NROW, P, BANDW]
        for i_t in range(NROW):
            gi = i_t + NROW * half
            strip = o[i_t]
            for lj in range(i_t, i_t + 9):
                gj = (lj + NROW * half) % 16
                V = strip[:, (lj - i_t) * P : (lj - i_t + 1) * P]
                scores[b, gi * P : (gi + 1) * P, gj * P : (gj + 1) * P] = V
                if gj != gi:
                    scores[b, gj * P : (gj + 1) * P, gi * P : (gi + 1) * P] = V.T
    return scores


def kernel(hidden_states, proj_w, proj_b, clf_w, clf_b):
    from concourse.bass_utils import run_bass_kernel_spmd

    nc = _get_nc()
    in_maps = _make_in_maps(hidden_states, proj_w, proj_b, clf_w, clf_b)
    res = run_bass_kernel_spmd(nc, in_maps, core_ids=list(range(NCORES)))
    return _assemble(res.results)


def run_traced(hidden_states, proj_w, proj_b, clf_w, clf_b):
    """Like kernel(), but also returns BassKernelResults with trace info."""
    from concourse.bass_utils import run_bass_kernel_spmd

    nc = _get_nc()
    in_maps = _make_in_maps(hidden_states, proj_w, proj_b, clf_w, clf_b)
    res = run_bass_kernel_spmd(
        nc, in_maps, core_ids=list(range(NCORES)), trace=True
    )
    return _assemble(res.results), res
